# revision 12
# baseline (speedup 1.0000x reference)
"""Trainium2 Bass kernel for nn_CellLayer (GRU over B=16, T=4096, D=256, H=512).

Strategy: chunk-parallel GRU with warmup ("fading memory" / DEER-style):
  - T=4096 split into C=64 chunks of L=64 steps; 8 chunks per NeuronCore.
  - Each core processes its 8 chunks x 16 batch = 128 independent sequences
    as the PSUM partition dim, stepping time sequentially for S = L + V slots.
  - Each chunk starts V steps early from h=0; contraction of the GRU makes the
    warmup error negligible (validated numerically).
  - Slots where a chunk's true time < 0 are masked to exact no-ops (zero x and
    masked biases keep h at exactly 0 until the chunk's true start).
  - Per step, all matmuls (hidden W_hh, input W_ih, bias rows) accumulate in 4
    PSUM banks (r / z / nh / ni); gate math on ACT+DVE; h' transposed via PE
    back into stationary layout for the next step. Matmul dtype float32r
    (TF32-like, full speed); master h state fp32.
"""

import os
import sys

sys.path.insert(0, "/opt/trn_rl_repo")

import numpy as np

import concourse.bass as bass
import concourse.mybir as mybir
import concourse.tile as tile
from concourse import bacc
from concourse.bass import ds, ts
from concourse.bass_utils import run_bass_kernel_spmd
from concourse.masks import make_identity

B, T, D, H = 16, 4096, 256, 512
G = 3 * H  # 1536 gate dims
NCORES = 8
C = 64  # total chunks
L = T // C  # 64 steps output per chunk
V = 32  # warmup steps (validated numerically: converged at V=24, f32r floor ~8e-5)
S = L + V  # slots per core
if os.environ.get("KERNEL_S_OVERRIDE"):  # dev: truncated build for fast iteration
    S = int(os.environ["KERNEL_S_OVERRIDE"])
BC = (C // NCORES) * B  # 128 partition lanes: (chunk_local, batch)
P = 128
DK = D // P  # 2 contract chunks for x
HK = H // P  # 4 contract chunks for h

F32 = mybir.dt.float32
F32R = mybir.dt.float32r

_cached = {}


def build_nc():
    nc = bacc.Bacc(None, target_bir_lowering=False)

    # ---- DRAM I/O (per-core values supplied via in_maps) ----
    # xs_t[s, :, bc]: x for slot s, transposed (d on first axis); zeros where masked
    xs_t = nc.declare_dram_parameter("xs_t", [S, D, BC], F32R, isOutput=False)
    # mask[s, bc]: 1.0 when slot s is active for lane bc's chunk, else 0.0
    mask = nc.declare_dram_parameter("mask", [S, BC], F32R, isOutput=False)
    # weights, pre-transposed on host: w_hh_t[h, g], w_ih_t[d, g]
    w_hh_t = nc.declare_dram_parameter("w_hh_t", [H, G], F32R, isOutput=False)
    w_ih_t = nc.declare_dram_parameter("w_ih_t", [D, G], F32R, isOutput=False)
    # bias rows: [b_r | b_z | b_in | b_n] each (512,) -> (1, 2048)
    brow = nc.declare_dram_parameter("brow", [1, G + H], F32R, isOutput=False)
    # output: ys[s', h, bc] for output slots s' = s - V (f32r == fp32 bits)
    ys = nc.declare_dram_parameter("ys", [L, BC, H], F32R, isOutput=True)

    with tile.TileContext(nc) as tc:
        _build_body(nc, tc, xs_t, mask, w_hh_t, w_ih_t, brow, ys)
    nc.compile()
    return nc


def _build_body(nc, tc, xs_t, mask, w_hh_t, w_ih_t, brow, ys):
    from contextlib import ExitStack

    ctx = ExitStack()
    with ctx:
        const = ctx.enter_context(tc.tile_pool(name="const", bufs=1))
        xpool = ctx.enter_context(tc.tile_pool(name="xpool", bufs=6))
        state = ctx.enter_context(tc.tile_pool(name="state", bufs=2))
        gates = ctx.enter_context(tc.tile_pool(name="gates", bufs=3))
        hout = ctx.enter_context(tc.tile_pool(name="hout", bufs=4))
        psum = ctx.enter_context(tc.tile_pool(name="psum", bufs=1, space="PSUM"))
        psum_t = ctx.enter_context(tc.tile_pool(name="psum_t", bufs=2, space="PSUM"))

        # ---- resident constants ----
        whh = const.tile([P, HK, G], F32R)  # [h%128, h//128, g]
        nc.sync.dma_start(whh[:], w_hh_t.rearrange("(hk p) g -> p hk g", p=P))
        wih = const.tile([P, DK, G], F32R)
        nc.sync.dma_start(wih[:], w_ih_t.rearrange("(dk p) g -> p dk g", p=P))
        brows = const.tile([1, G + H], F32R)
        nc.sync.dma_start(brows[:], brow[:])
        masks = const.tile([1, S, BC], F32R)
        nc.sync.dma_start(masks[:], mask.rearrange("s b -> (s b)").rearrange("(o sb) -> o sb", o=1).rearrange("o (s b) -> o s b", s=S))
        ident = const.tile([P, P], F32)
        make_identity(nc, ident[:])
        identr = const.tile([P, P], F32R)
        nc.vector.tensor_copy(identr[:], ident[:])

        # ---- state: hT (stationary, f32r) and h (fp32 master) ----
        hT = state.tile([P, HK, BC], F32R, name="hT")  # [h%128, h//128, bc]
        h = state.tile([BC, H], F32R, name="h")
        nc.vector.memset(hT[:].bitcast(F32), 0.0)
        nc.vector.memset(h[:].bitcast(F32), 0.0)

        for s in range(S):
            # x tile for this slot
            xt = xpool.tile([P, DK, BC], F32R, name="xt")
            nc.sync.dma_start(xt[:], xs_t[s].rearrange("(dk p) b -> p dk b", p=P))

            # ---- PSUM accumulation: gates = h @ WhhT + x @ WihT + mask*b ----
            pr = psum.tile([BC, H], F32, name="pr")
            pz = psum.tile([BC, H], F32, name="pz")
            pnh = psum.tile([BC, H], F32, name="pnh")
            pni = psum.tile([BC, H], F32, name="pni")
            mcol = masks[:, s, :]  # (1, BC)

            # r bank: 4 h-chunks + 2 x-chunks + bias
            for j in range(HK):
                nc.tensor.matmul(pr[:], hT[:, j], whh[:, j, 0:H], start=(j == 0), stop=False)
            for k in range(DK):
                nc.tensor.matmul(pr[:], xt[:, k], wih[:, k, 0:H], start=False, stop=False)
            nc.tensor.matmul(pr[:], mcol, brows[:, 0:H], start=False, stop=True)

            # z bank
            for j in range(HK):
                nc.tensor.matmul(pz[:], hT[:, j], whh[:, j, H : 2 * H], start=(j == 0), stop=False)
            for k in range(DK):
                nc.tensor.matmul(pz[:], xt[:, k], wih[:, k, H : 2 * H], start=False, stop=False)
            nc.tensor.matmul(pz[:], mcol, brows[:, H : 2 * H], start=False, stop=True)

            # nh bank: h-side of n gate + b_n
            for j in range(HK):
                nc.tensor.matmul(pnh[:], hT[:, j], whh[:, j, 2 * H : 3 * H], start=(j == 0), stop=False)
            nc.tensor.matmul(pnh[:], mcol, brows[:, G : G + H], start=False, stop=True)

            # ni bank: x-side of n gate + b_in
            for k in range(DK):
                nc.tensor.matmul(pni[:], xt[:, k], wih[:, k, 2 * H : 3 * H], start=(k == 0), stop=False)
            nc.tensor.matmul(pni[:], mcol, brows[:, 2 * H : 3 * H], start=False, stop=True)

            # ---- gate math ----
            r = gates.tile([BC, H], F32, name="r")
            nc.scalar.activation(r[:], pr[:], mybir.ActivationFunctionType.Sigmoid)
            z = gates.tile([BC, H], F32, name="z")
            nc.scalar.activation(z[:], pz[:], mybir.ActivationFunctionType.Sigmoid)

            t2 = gates.tile([BC, H], F32, name="t2")
            nc.vector.tensor_tensor(t2[:], pnh[:], r[:], mybir.AluOpType.mult)
            t3 = gates.tile([BC, H], F32, name="t3")
            nc.vector.tensor_tensor(t3[:], t2[:], pni[:], mybir.AluOpType.add)
            n = gates.tile([BC, H], F32, name="n")
            nc.scalar.activation(n[:], t3[:], mybir.ActivationFunctionType.Tanh)

            # h' = n + z*(h - n) = z*h - (z-1)*n
            u = gates.tile([BC, H], F32, name="u")
            nc.vector.tensor_tensor(u[:], z[:], h[:], mybir.AluOpType.mult)
            v = gates.tile([BC, H], F32, name="v")
            nc.vector.scalar_tensor_tensor(
                v[:], z[:], 1.0, n[:], mybir.AluOpType.subtract, mybir.AluOpType.mult
            )
            hnew = hout.tile([BC, H], F32R, name="hnew")
            nc.vector.tensor_tensor(hnew[:], u[:], v[:], mybir.AluOpType.subtract)
            h = hnew

            # ---- transpose h' back to stationary layout for next step ----
            # Normal-mode matmul against identity: out[h, bc] = sum_b h'[b, h] * I[b, bc].
            # (PE transpose-mode doesn't count as activity for the HAM clock
            # gate and was keeping the PE throttled at 1.2 GHz.)
            if s != S - 1:
                pT = psum_t.tile([P, HK * P], F32, name="pT")
                for j in range(HK):
                    nc.tensor.matmul(
                        pT[:, ts(j, P)], hnew[:, ts(j, P)], identr[:], start=True, stop=True
                    )
                hT = state.tile([P, HK, BC], F32R, name="hT")
                for j in range(HK):
                    if j % 2 == 0:
                        nc.scalar.activation(
                            hT[:, j], pT[:, ts(j, P)], mybir.ActivationFunctionType.Copy
                        )
                    else:
                        nc.vector.tensor_copy(hT[:, j], pT[:, ts(j, P)])

            # ---- output ----
            if s >= V:
                nc.sync.dma_start(ys[s - V], hnew[:])


def _prep_inputs(xs, W_ih, W_hh, b, b_n):
    """Build per-core input maps."""
    xs = np.ascontiguousarray(xs, dtype=np.float32)
    w_hh_t = np.ascontiguousarray(W_hh.T, dtype=np.float32)  # (H, G)
    w_ih_t = np.ascontiguousarray(W_ih.T, dtype=np.float32)  # (D, G)
    brow = np.concatenate([b, b_n]).reshape(1, G + H).astype(np.float32)

    in_maps = []
    for core in range(NCORES):
        xs_t = np.zeros((S, D, BC), np.float32)
        m = np.zeros((S, BC), np.float32)
        for cl in range(C // NCORES):
            c = core * (C // NCORES) + cl
            lanes = slice(cl * B, (cl + 1) * B)
            t0 = c * L - V  # true time of slot 0
            lo_s = max(0, -t0)  # first active slot
            t_lo = t0 + lo_s
            t_hi = min((c + 1) * L, t0 + S)  # min() only binds under S override
            # xs[b, t, :] -> xs_t[s, d, lane]
            blk = xs[:, t_lo:t_hi, :]  # (B, nt, D)
            xs_t[lo_s : lo_s + (t_hi - t_lo), :, lanes] = blk.transpose(1, 2, 0)
            m[lo_s:, lanes] = 1.0
        in_maps.append({"xs_t": xs_t, "mask": m, "w_hh_t": w_hh_t, "w_ih_t": w_ih_t, "brow": brow})
    return in_maps


def kernel(xs, W_ih, W_hh, b, b_n):
    xs = np.asarray(xs, dtype=np.float32)
    if "nc" not in _cached:
        _cached["nc"] = build_nc()
    nc = _cached["nc"]
    in_maps = _prep_inputs(xs, W_ih, W_hh, b, b_n)
    res = run_bass_kernel_spmd(nc, in_maps, core_ids=list(range(NCORES)))
    _cached["last_results"] = res
    # assemble (B, T, H)
    ys = np.empty((B, T, H), np.float32)
    for core in range(NCORES):
        out = res.results[core]["ys"]  # (L, BC, H)
        for cl in range(C // NCORES):
            c = core * (C // NCORES) + cl
            lanes = slice(cl * B, (cl + 1) * B)
            # out[s', lane, :] -> ys[b, c*L + s', :]
            ys[:, c * L : (c + 1) * L, :] = out[:, lanes, :].transpose(1, 0, 2)
    return ys


# revision 15
# speedup vs baseline: 1.0505x; 1.0505x over previous
"""Trainium2 Bass kernel for nn_CellLayer (GRU over B=16, T=4096, D=256, H=512).

Strategy: chunk-parallel GRU with warmup ("fading memory" / DEER-style):
  - T=4096 split into C=64 chunks of L=64 steps; 8 chunks per NeuronCore.
  - Each core processes its 8 chunks x 16 batch = 128 independent sequences
    as the PSUM partition dim, stepping time sequentially for S = L + V slots.
  - Each chunk starts V steps early from h=0; contraction of the GRU makes the
    warmup error negligible (validated numerically).
  - Slots where a chunk's true time < 0 are masked to exact no-ops (zero x and
    masked biases keep h at exactly 0 until the chunk's true start).
  - Per step, all matmuls (hidden W_hh, input W_ih, bias rows) accumulate in 4
    PSUM banks (r / z / nh / ni); gate math on ACT+DVE; h' transposed via PE
    back into stationary layout for the next step. Matmul dtype float32r
    (TF32-like, full speed); master h state fp32.
"""

import os
import sys

sys.path.insert(0, "/opt/trn_rl_repo")

import numpy as np

import concourse.bass as bass
import concourse.mybir as mybir
import concourse.tile as tile
from concourse import bacc
from concourse.bass import ds, ts
from concourse.bass_utils import run_bass_kernel_spmd
from concourse.masks import make_identity

B, T, D, H = 16, 4096, 256, 512
G = 3 * H  # 1536 gate dims
NCORES = 8
C = 64  # total chunks
L = T // C  # 64 steps output per chunk
V = 32  # warmup steps (validated numerically: converged at V=24, f32r floor ~8e-5)
S = L + V  # slots per core
if os.environ.get("KERNEL_S_OVERRIDE"):  # dev: truncated build for fast iteration
    S = int(os.environ["KERNEL_S_OVERRIDE"])
BC = (C // NCORES) * B  # 128 partition lanes: (chunk_local, batch)
P = 128
DK = D // P  # 2 contract chunks for x
HK = H // P  # 4 contract chunks for h

F32 = mybir.dt.float32
F32R = mybir.dt.float32r

_cached = {}


def build_nc():
    nc = bacc.Bacc(None, target_bir_lowering=False)

    # ---- DRAM I/O (per-core values supplied via in_maps) ----
    # xs_t[s, :, bc]: x for slot s, transposed (d on first axis); zeros where masked
    xs_t = nc.declare_dram_parameter("xs_t", [S, D, BC], F32R, isOutput=False)
    # mask[s, bc]: 1.0 when slot s is active for lane bc's chunk, else 0.0
    mask = nc.declare_dram_parameter("mask", [S, BC], F32R, isOutput=False)
    # weights, pre-transposed on host: w_hh_t[h, g], w_ih_t[d, g]
    w_hh_t = nc.declare_dram_parameter("w_hh_t", [H, G], F32R, isOutput=False)
    w_ih_t = nc.declare_dram_parameter("w_ih_t", [D, G], F32R, isOutput=False)
    # bias rows: [b_r | b_z | b_in | b_n] each (512,) -> (1, 2048)
    brow = nc.declare_dram_parameter("brow", [1, G + H], F32R, isOutput=False)
    # output: ys[s', h, bc] for output slots s' = s - V (f32r == fp32 bits)
    ys = nc.declare_dram_parameter("ys", [L, BC, H], F32R, isOutput=True)

    with tile.TileContext(nc) as tc:
        _build_body(nc, tc, xs_t, mask, w_hh_t, w_ih_t, brow, ys)
    nc.compile()
    return nc


def _build_body(nc, tc, xs_t, mask, w_hh_t, w_ih_t, brow, ys):
    from contextlib import ExitStack

    ctx = ExitStack()
    with ctx:
        const = ctx.enter_context(tc.tile_pool(name="const", bufs=1))
        xpool = ctx.enter_context(tc.tile_pool(name="xpool", bufs=6))
        state = ctx.enter_context(tc.tile_pool(name="state", bufs=2))
        gates = ctx.enter_context(tc.tile_pool(name="gates", bufs=3))
        hout = ctx.enter_context(tc.tile_pool(name="hout", bufs=4))
        psum = ctx.enter_context(tc.tile_pool(name="psum", bufs=1, space="PSUM"))
        psum_t = ctx.enter_context(tc.tile_pool(name="psum_t", bufs=2, space="PSUM"))

        # ---- resident constants ----
        whh = const.tile([P, HK, G], F32R)  # [h%128, h//128, g]
        nc.sync.dma_start(whh[:], w_hh_t.rearrange("(hk p) g -> p hk g", p=P))
        wih = const.tile([P, DK, G], F32R)
        nc.sync.dma_start(wih[:], w_ih_t.rearrange("(dk p) g -> p dk g", p=P))
        brows = const.tile([1, G + H], F32R)
        nc.sync.dma_start(brows[:], brow[:])
        masks = const.tile([1, S, BC], F32R)
        nc.sync.dma_start(masks[:], mask.rearrange("s b -> (s b)").rearrange("(o sb) -> o sb", o=1).rearrange("o (s b) -> o s b", s=S))
        ident = const.tile([P, P], F32)
        make_identity(nc, ident[:])
        identr = const.tile([P, P], F32R)
        nc.vector.tensor_copy(identr[:], ident[:])

        # ---- state: hT (stationary, f32r) and h (master, 2 half tiles) ----
        HH = H // 2
        hT = state.tile([P, HK, BC], F32R, name="hT")  # [h%128, h//128, bc]
        h0 = state.tile([BC, HH], F32R, name="h0")
        h1 = state.tile([BC, HH], F32R, name="h1")
        nc.vector.memset(hT[:].bitcast(F32), 0.0)
        nc.vector.memset(h0[:].bitcast(F32), 0.0)
        nc.vector.memset(h1[:].bitcast(F32), 0.0)
        hhalves = [h0, h1]

        for s in range(S):
            # x tile for this slot
            xt = xpool.tile([P, DK, BC], F32R, name="xt")
            nc.sync.dma_start(xt[:], xs_t[s].rearrange("(dk p) b -> p dk b", p=P))

            # ---- PSUM accumulation: gates = h @ WhhT + x @ WihT + mask*b ----
            pr = psum.tile([BC, H], F32, name="pr")
            pz = psum.tile([BC, H], F32, name="pz")
            pnh = psum.tile([BC, H], F32, name="pnh")
            pni = psum.tile([BC, H], F32, name="pni")
            mcol = masks[:, s, :]  # (1, BC)

            # r bank: 4 h-chunks + 2 x-chunks + bias
            for j in range(HK):
                nc.tensor.matmul(pr[:], hT[:, j], whh[:, j, 0:H], start=(j == 0), stop=False)
            for k in range(DK):
                nc.tensor.matmul(pr[:], xt[:, k], wih[:, k, 0:H], start=False, stop=False)
            nc.tensor.matmul(pr[:], mcol, brows[:, 0:H], start=False, stop=True)

            # z bank
            for j in range(HK):
                nc.tensor.matmul(pz[:], hT[:, j], whh[:, j, H : 2 * H], start=(j == 0), stop=False)
            for k in range(DK):
                nc.tensor.matmul(pz[:], xt[:, k], wih[:, k, H : 2 * H], start=False, stop=False)
            nc.tensor.matmul(pz[:], mcol, brows[:, H : 2 * H], start=False, stop=True)

            # nh bank: h-side of n gate + b_n
            for j in range(HK):
                nc.tensor.matmul(pnh[:], hT[:, j], whh[:, j, 2 * H : 3 * H], start=(j == 0), stop=False)
            nc.tensor.matmul(pnh[:], mcol, brows[:, G : G + H], start=False, stop=True)

            # ni bank: x-side of n gate + b_in
            for k in range(DK):
                nc.tensor.matmul(pni[:], xt[:, k], wih[:, k, 2 * H : 3 * H], start=(k == 0), stop=False)
            nc.tensor.matmul(pni[:], mcol, brows[:, 2 * H : 3 * H], start=False, stop=True)

            # ---- gate math, half-split (k = 0,1 over 256-wide halves) so the
            # serial chain pipelines across ACT/DVE/GPSIMD and PE stays fed ----
            newh = []
            pT = psum_t.tile([P, HK * P], F32R, name="pT") if s != S - 1 else None
            for k in range(2):
                hs = ds(k * HH, HH)
                rk = gates.tile([BC, HH], F32, name=f"r{k}")
                nc.scalar.activation(rk[:], pr[:, hs], mybir.ActivationFunctionType.Sigmoid)
                zk = gates.tile([BC, HH], F32, name=f"z{k}")
                nc.scalar.activation(zk[:], pz[:, hs], mybir.ActivationFunctionType.Sigmoid)
                # u = z*h on GPSIMD (off the critical path, idle engine)
                uk = gates.tile([BC, HH], F32, name=f"u{k}")
                nc.gpsimd.tensor_tensor(uk[:], zk[:], hhalves[k][:], mybir.AluOpType.mult)
                t2k = gates.tile([BC, HH], F32, name=f"t2{k}")
                nc.vector.tensor_tensor(t2k[:], pnh[:, hs], rk[:], mybir.AluOpType.mult)
                t3k = gates.tile([BC, HH], F32, name=f"t3{k}")
                nc.vector.tensor_tensor(t3k[:], t2k[:], pni[:, hs], mybir.AluOpType.add)
                nk = gates.tile([BC, HH], F32, name=f"n{k}")
                nc.scalar.activation(nk[:], t3k[:], mybir.ActivationFunctionType.Tanh)
                # h' = z*h - (z-1)*n
                vk = gates.tile([BC, HH], F32, name=f"v{k}")
                nc.vector.scalar_tensor_tensor(
                    vk[:], zk[:], 1.0, nk[:], mybir.AluOpType.subtract, mybir.AluOpType.mult
                )
                hk = hout.tile([BC, HH], F32R, name=f"hnew{k}")
                nc.vector.tensor_tensor(hk[:], uk[:], vk[:], mybir.AluOpType.subtract)
                newh.append(hk)

                # transpose this half's two 128-wide j-slices for next step
                if s != S - 1:
                    for jj in range(2):
                        j = 2 * k + jj
                        nc.tensor.transpose(
                            pT[:, ts(j, P)], hk[:, ts(jj, P)], identr[:]
                        )

                if s >= V:
                    nc.sync.dma_start(ys[s - V, :, hs], hk[:])

            hhalves = newh
            if s != S - 1:
                hT = state.tile([P, HK, BC], F32R, name="hT")
                for j in range(HK):
                    nc.vector.tensor_copy(hT[:, j], pT[:, ts(j, P)])


def _prep_inputs(xs, W_ih, W_hh, b, b_n):
    """Build per-core input maps."""
    xs = np.ascontiguousarray(xs, dtype=np.float32)
    w_hh_t = np.ascontiguousarray(W_hh.T, dtype=np.float32)  # (H, G)
    w_ih_t = np.ascontiguousarray(W_ih.T, dtype=np.float32)  # (D, G)
    brow = np.concatenate([b, b_n]).reshape(1, G + H).astype(np.float32)

    in_maps = []
    for core in range(NCORES):
        xs_t = np.zeros((S, D, BC), np.float32)
        m = np.zeros((S, BC), np.float32)
        for cl in range(C // NCORES):
            c = core * (C // NCORES) + cl
            lanes = slice(cl * B, (cl + 1) * B)
            t0 = c * L - V  # true time of slot 0
            lo_s = max(0, -t0)  # first active slot
            t_lo = t0 + lo_s
            t_hi = min((c + 1) * L, t0 + S)  # min() only binds under S override
            # xs[b, t, :] -> xs_t[s, d, lane]
            blk = xs[:, t_lo:t_hi, :]  # (B, nt, D)
            xs_t[lo_s : lo_s + (t_hi - t_lo), :, lanes] = blk.transpose(1, 2, 0)
            m[lo_s:, lanes] = 1.0
        in_maps.append({"xs_t": xs_t, "mask": m, "w_hh_t": w_hh_t, "w_ih_t": w_ih_t, "brow": brow})
    return in_maps


def kernel(xs, W_ih, W_hh, b, b_n):
    xs = np.asarray(xs, dtype=np.float32)
    if "nc" not in _cached:
        _cached["nc"] = build_nc()
    nc = _cached["nc"]
    in_maps = _prep_inputs(xs, W_ih, W_hh, b, b_n)
    res = run_bass_kernel_spmd(nc, in_maps, core_ids=list(range(NCORES)))
    _cached["last_results"] = res
    # assemble (B, T, H)
    ys = np.empty((B, T, H), np.float32)
    for core in range(NCORES):
        out = res.results[core]["ys"]  # (L, BC, H)
        for cl in range(C // NCORES):
            c = core * (C // NCORES) + cl
            lanes = slice(cl * B, (cl + 1) * B)
            # out[s', lane, :] -> ys[b, c*L + s', :]
            ys[:, c * L : (c + 1) * L, :] = out[:, lanes, :].transpose(1, 0, 2)
    return ys
